# revision 32
# baseline (speedup 1.0000x reference)
"""CFConv (gnn message passing) Trainium2 kernel.

Math (per batch b):
    f1 = ssp(r @ W1 + b1)            ssp(x) = softplus(x) - log2
    f2 = ssp(f1 @ W2 + b2)
    out[i, d] = sum_j x[j, d] * f2[i, j, d]

Sharding: data-parallel over batch B=8 across the 8 cores (one batch each).

The kernel is ACT-bound: softplus = Ln(Exp(.)+1) costs two table passes
per element on the scalar engine (1 elem/lane/cycle at 1.2 GHz, no native
softplus table on this stack), twice per element = ~218 us/core of pure
ACT work.  Everything else (PE matmuls, DVE contraction, DMA) overlaps
under it.

Per-core device pipeline (features on partitions, (i,j) on the free dim):
  1. r is pre-transposed on the host to [128, PAIRS] bf16 (partitions
     0:64 hold the rbf features of even j, 64:128 of odd j), so the
     device does plain contiguous DMA loads (no xbar transpose).
  2. mm1: two K=64 row-tiled matmuls against a stacked [W1;W1] stationary
     (they run concurrently in different PE row groups).
  3. act1: Exp per 2048-col PSUM group then Ln per chunk; the "- log2"
     shift is folded into layer 2's bias (b2' below).  One of the four
     groups per chunk computes Exp on the otherwise-idle DVE instead,
     via the Schraudolph bits trick in bf16 space (one fused
     tensor_scalar: bits_i16 = round(A16*z1 + (A16*b1+B16)); the bit
     pattern IS exp(z1+b1) in bf16).  The exact ACT Ln that follows
     damps the sawtooth error by sigmoid(z1) and layer 2 stays exact:
     measured end-to-end rel err 0.0146 (budget 2e-2, fixed input seed).
     This takes a quarter of the Exp1 pass off the ACT bottleneck; the
     trick group runs last in the PSUM pool chain so its DVE convert
     hides under the Exp2 block and never stalls ACT.
  4. mm2: K=128 matmuls against W2.
  5. act2: exact two-pass softplus, bias b2' = b2 - log2 * sum_d W2[d,:].
  6. Contraction over j on DVE: prod = a2 * xT (xT broadcast over i),
     bf16 pair-halving add (TT at 2x) then 3D tensor_reduce over the
     inner j axis (1x), even+odd add, then a per-partition add of the
     "- log2 * sum_j x[j,d]" correction.  Output stays [d, i] on device;
     the host transposes back.

Scheduling: layer-2 work of chunk c-1 interleaves with layer-1 work of
chunk c at group granularity.  PSUM parity: chunk c uses pool c%2 for
both its layers, so neither matmul stream waits on the other chunk's Exp
reads.  Ln1(c) is emitted before Ln2(c-1) so mm2(c) starts under
Ln2(c-1).  For timing loops (reps > 1), eight full problem passes are
emitted per hardware-loop iteration and CHAINED (body k+1's chunk 0
interleaves with body k's last-chunk layer 2), so the per-iteration
all-engine barrier and the drain/ramp amortize over eight passes.
"""

import os
import tempfile

# The neuronxcc NEFF cache keys on the jit module interface, not the BIR
# content, so same-interface programs from other runs can serve stale
# NEFFs.  Give every process its own empty cache (compiles are ~2 s).
os.environ.setdefault(
    "NEURON_COMPILE_CACHE_URL", tempfile.mkdtemp(prefix="neff-cache-"))

import numpy as np
import ml_dtypes

import concourse.bass as bass
import concourse.tile as tile
from concourse import bacc, mybir
from concourse.bass_utils import run_bass_kernel_spmd

LOG2 = float(np.log(2.0))

B, N, D, RBF = 8, 256, 128, 64
PAIRS = N * N // 2            # 32768 row-pairs per batch
CHUNK_PAIRS = 4096            # pairs per DMA chunk (1 MiB)
GROUP_PAIRS = 1024            # pairs per PSUM group (8 query nodes i)
SUB = 512                     # pairs per matmul (one PSUM bank)
I_PER_GROUP = GROUP_PAIRS // (N // 2)   # 8
H = CHUNK_PAIRS // GROUP_PAIRS          # groups per chunk tile (4)
N_CORES = 8

BF16 = mybir.dt.bfloat16
F32 = mybir.dt.float32
I16 = mybir.dt.int16

# Schraudolph bf16-bits exp constants for the half-trick Exp1 path
A16 = 2.0 ** 7 / LOG2
C16 = 0.0579 * 2 ** 7
B16 = 127.0 * 2 ** 7 - C16


def _build_program(reps: int = 1):
    # Restrict the ACT-table chooser to the one set holding BOTH Exp and Ln;
    # otherwise it alternates between per-function sets and pays a ~2.7us
    # table load on every activation.
    import concourse.bacc as _bacc_mod
    from concourse.hw_specs import get_activation_tables as _gat
    _orig = _gat("gen3")
    _both = mybir.ActivationFunctionType.Exp, mybir.ActivationFunctionType.Ln
    _patched = {
        name: (funcs if name == "natural_log_exp_and_others"
               else type(funcs)(f for f in funcs if f not in _both))
        for name, funcs in _orig.items()
    }
    _bacc_mod.get_activation_tables = lambda arch: _patched

    nc = bacc.Bacc("TRN2", target_bir_lowering=False, debug=False,
                   num_devices=N_CORES)

    rpt = nc.dram_tensor("rpt", [2 * RBF, PAIRS], BF16, kind="ExternalInput").ap()
    xte = nc.dram_tensor("xte", [D, N // 2], BF16, kind="ExternalInput").ap()
    xto = nc.dram_tensor("xto", [D, N // 2], BF16, kind="ExternalInput").ap()
    corr = nc.dram_tensor("corr", [D, 1], F32, kind="ExternalInput").ap()
    w1s = nc.dram_tensor("w1s", [2 * RBF, D], BF16, kind="ExternalInput").ap()
    w2 = nc.dram_tensor("w2", [D, D], BF16, kind="ExternalInput").ap()
    b1 = nc.dram_tensor("b1c", [D, 1], F32, kind="ExternalInput").ap()
    b1s = nc.dram_tensor("b1s", [D, 1], F32, kind="ExternalInput").ap()
    b2p = nc.dram_tensor("b2p", [D, 1], F32, kind="ExternalInput").ap()
    outT = nc.dram_tensor("outT", [D, N], F32, kind="ExternalOutput").ap()

    f_exp = mybir.ActivationFunctionType.Exp
    f_ln = mybir.ActivationFunctionType.Ln
    mult = mybir.AluOpType.mult
    add = mybir.AluOpType.add

    with tile.TileContext(nc) as tc:
        with (
            tc.tile_pool(name="const", bufs=1) as const,
            tc.tile_pool(name="rt", bufs=4) as rt_pool,
            tc.tile_pool(name="e1", bufs=1) as e1_pool,
            tc.tile_pool(name="e2", bufs=1) as e2_pool,
            tc.tile_pool(name="a1", bufs=2) as a1_pool,
            tc.tile_pool(name="a2", bufs=2) as a2_pool,
            tc.tile_pool(name="prod", bufs=2) as prod_pool,
            tc.tile_pool(name="acc", bufs=2) as acc_pool,
            tc.tile_pool(name="osb", bufs=1) as out_pool,
            tc.tile_pool(name="f1", bufs=1, space="PSUM") as f1_pool,
            tc.tile_pool(name="f2", bufs=1, space="PSUM") as f2_pool,
        ):
            w1s_t = const.tile([2 * RBF, D], BF16, tag="w1s")
            w2_t = const.tile([D, D], BF16, tag="w2")
            xte_t = const.tile([D, N // 2], BF16, tag="xte")
            xto_t = const.tile([D, N // 2], BF16, tag="xto")
            b1_t = const.tile([D, 1], F32, tag="b1")
            b1s_t = const.tile([D, 1], F32, tag="b1s")
            b2p_t = const.tile([D, 1], F32, tag="b2p")
            corr_t = const.tile([D, 1], F32, tag="corr")
            # w1s/b1 first: they gate mm1/Exp1 of the first group.  The
            # first rt quarter-DMA is issued right after them (inside
            # body()) so it isn't queued behind the remaining consts.
            nc.sync.dma_start(w1s_t[:], w1s[:])
            nc.sync.dma_start(b1_t[:], b1[:])
            nc.sync.dma_start(b1s_t[:], b1s[:])

            out_sb = out_pool.tile([D, N], F32, tag="osb")

            def load_rest_of_consts():
                nc.sync.dma_start(w2_t[:], w2[:])
                nc.sync.dma_start(xte_t[:], xte[:])
                nc.sync.dma_start(xto_t[:], xto[:])
                nc.sync.dma_start(b2p_t[:], b2p[:])
                nc.sync.dma_start(corr_t[:], corr[:])

            # Tiny warmup activation right after the const loads: hoists the
            # ~2.7us ACT table load to t~0 where it overlaps the first DMA
            # instead of sitting in front of the first real Exp.
            warm = acc_pool.tile([D, 1], F32, tag="warm")
            nc.scalar.activation(warm[:], b1_t[:],
                                 mybir.ActivationFunctionType.Exp, bias=0.0)

            jw = N // 2
            G2 = 2 * GROUP_PAIRS          # cols per group (2048)
            PW = H * G2                   # cols per chunk tile (8192)
            I_PAIR = H * I_PER_GROUP      # 32 query nodes per chunk tile
            xe4 = xte_t[:, None, None, :].broadcast_to([D, H, I_PER_GROUP, jw])
            xo4 = xto_t[:, None, None, :].broadcast_to([D, H, I_PER_GROUP, jw])
            xe1 = xte_t[:, None, None, :].broadcast_to([D, 1, I_PER_GROUP, jw])
            xo1 = xto_t[:, None, None, :].broadcast_to([D, 1, I_PER_GROUP, jw])

            def stage1_half(rt, e1w, h, pool):
                """mm1 + Exp for one group (half pair)."""
                g0 = h * GROUP_PAIRS
                # f1 layout: [even 0:GROUP | odd GROUP:2*GROUP]
                f1 = pool.tile([D, G2], F32, tag="ps")
                for s in range(GROUP_PAIRS // SUB):
                    cs = g0 + s * SUB
                    nc.tensor.matmul(
                        f1[:, s * SUB:(s + 1) * SUB],
                        w1s_t[0:RBF, :],
                        rt[0:RBF, cs:cs + SUB],
                    )
                for s in range(GROUP_PAIRS // SUB):
                    cs = g0 + s * SUB
                    nc.tensor.matmul(
                        f1[:, GROUP_PAIRS + s * SUB:
                            GROUP_PAIRS + (s + 1) * SUB],
                        w1s_t[RBF:2 * RBF, :],
                        rt[RBF:2 * RBF, cs:cs + SUB],
                    )
                # softplus(z1 + b1) = Ln(Exp(z1 + b1) + 1).  Odd groups
                # compute Exp approximately on the (otherwise idle) DVE via
                # the Schraudolph bits trick in bf16 space -- one fused
                # tensor_scalar: bits_i16 = round(A16*z1 + (A16*b1 + B16)),
                # whose bit pattern IS exp(z1+b1) in bf16.  The exact ACT Ln
                # that follows damps the sawtooth error by sigmoid(z1), and
                # layer 2 stays exact; measured end-to-end rel err 0.0172
                # (budget 2e-2).  This moves half of Exp1 off the ACT
                # bottleneck (~15 us/body).
                if h == 3:
                    nc.vector.tensor_scalar(
                        out=e1w[:, h * G2:(h + 1) * G2].bitcast(I16),
                        in0=f1[:],
                        scalar1=float(A16),
                        scalar2=b1s_t[:],
                        op0=mult,
                        op1=add,
                    )
                else:
                    nc.scalar.activation(
                        e1w[:, h * G2:(h + 1) * G2], f1[:], f_exp,
                        bias=b1_t[:])

            def stage2_half(a1w, e2w, h, pool):
                """mm2 + Exp for one group of the previous chunk."""
                f2 = pool.tile([D, G2], F32, tag="ps")
                for s in range(G2 // SUB):
                    nc.tensor.matmul(
                        f2[:, s * SUB:(s + 1) * SUB],
                        w2_t[:],
                        a1w[:, h * G2 + s * SUB:h * G2 + (s + 1) * SUB],
                    )
                nc.scalar.activation(
                    e2w[:, h * G2:(h + 1) * G2], f2[:], f_exp, bias=b2p_t[:])

            def contract(a2w, i0, h0, nh):
                """weighted j-reduction of groups [h0, h0+nh) of a chunk.

                a2w cols = [h: [even 1024 | odd 1024]] * H; writes
                out_sb[:, i0 + h0*I_PER_GROUP : ... + nh*I_PER_GROUP].
                """
                cw = nh * G2
                c0 = h0 * G2
                iw = nh * I_PER_GROUP
                xe = xe4 if nh == H else xe1
                xo = xo4 if nh == H else xo1
                if nh not in (1, H):
                    xe = xte_t[:, None, None, :].broadcast_to(
                        [D, nh, I_PER_GROUP, jw])
                    xo = xto_t[:, None, None, :].broadcast_to(
                        [D, nh, I_PER_GROUP, jw])
                prod = prod_pool.tile([D, PW], BF16, tag="prod")
                p4 = prod[:, c0:c0 + cw].rearrange(
                    "p (h par k j) -> p h par k j", h=nh, par=2, j=jw)
                a4 = a2w[:, c0:c0 + cw].rearrange(
                    "p (h par k j) -> p h par k j", h=nh, par=2, j=jw)
                nc.vector.tensor_tensor(
                    p4[:, :, 0, :, :], a4[:, :, 0, :, :], xe, mult)
                nc.vector.tensor_tensor(
                    p4[:, :, 1, :, :], a4[:, :, 1, :, :], xo, mult)
                # Pair-halve in bf16 (TT runs 2x) before the 1x TensorReduce:
                # halves the reduce input, ~2.3us/chunk less DVE time.
                m = 2 * iw      # 16 per group: (par, k)
                ph = prod_pool.tile([D, PW // 2], BF16, tag="ph")
                p3 = prod[:, c0:c0 + cw].rearrange(
                    "p (m half j) -> p m half j", m=m, half=2)
                nc.vector.tensor_tensor(
                    ph[:, 0:cw // 2].rearrange("p (m j) -> p m j", m=m),
                    p3[:, :, 0, :], p3[:, :, 1, :], add)
                # sums index m = h*16 + par*8 + k
                sums = acc_pool.tile([D, 2 * I_PAIR], F32, tag="sums")
                nc.vector.tensor_reduce(
                    sums[:, 0:2 * iw],
                    ph[:, 0:cw // 2].rearrange("p (m j) -> p m j", m=m),
                    axis=mybir.AxisListType.X,
                    op=add,
                )
                tmp = acc_pool.tile([D, I_PAIR], F32, tag="tmp")
                s4 = sums[:, 0:2 * iw].rearrange(
                    "p (h par k) -> p h par k", h=nh, par=2)
                nc.vector.tensor_add(
                    tmp[:, 0:iw].rearrange("p (h k) -> p h k", h=nh),
                    s4[:, :, 0, :], s4[:, :, 1, :])
                nc.vector.tensor_scalar_add(
                    out_sb[:, i0 + h0 * I_PER_GROUP:
                           i0 + h0 * I_PER_GROUP + iw],
                    tmp[:, 0:iw], corr_t[:])

            # Software-pipelined emission interleaving halves of chunk c's
            # layer 1 with halves of chunk c-1's layer 2, so every ACT op has
            # a PE window in front of it and ACT never head-of-line blocks.
            # PSUM parity: chunk c uses pool c%2 for BOTH its layers (its
            # layer 2 runs interleaved with chunk c+1's layer 1, which owns
            # the other pool), so neither matmul stream ever waits on the
            # other chunk's Exp reads.
            pools = (f1_pool, f2_pool)

            n_chunks = PAIRS // CHUNK_PAIRS

            def body(pending):
                """One full problem pass.  `pending` is the (a1w, i0, pool)
                of the previous body's last chunk: its layer 2 interleaves
                with this body's chunk 0 exactly like an ordinary chunk
                boundary, so chained bodies pipeline seamlessly."""
                first = pending is None
                for c in range(n_chunks):
                    rt = rt_pool.tile([2 * RBF, CHUNK_PAIRS], BF16, tag="rt")
                    if first and c == 0:
                        # Quarter the first load so mm1 of group 0 can
                        # start as soon as the first 256 KiB lands.  The
                        # remaining const DMAs queue behind quarter 0.
                        q = CHUNK_PAIRS // 4
                        nc.sync.dma_start(rt[:, 0:q], rpt[:, 0:q])
                        load_rest_of_consts()
                        for k in range(1, 4):
                            nc.sync.dma_start(
                                rt[:, k * q:(k + 1) * q],
                                rpt[:, k * q:(k + 1) * q],
                            )
                    elif first and c == 1:
                        hq = CHUNK_PAIRS // 2
                        for k in range(2):
                            nc.sync.dma_start(
                                rt[:, k * hq:(k + 1) * hq],
                                rpt[:, c * CHUNK_PAIRS + k * hq:
                                    c * CHUNK_PAIRS + (k + 1) * hq],
                            )
                    else:
                        nc.sync.dma_start(
                            rt[:],
                            rpt[:, c * CHUNK_PAIRS:(c + 1) * CHUNK_PAIRS],
                        )
                    # e1 (chunk c) and e2 (previous chunk) share ONE tile;
                    # their Ln passes run back to back from it.
                    ew = e1_pool.tile([D, 2 * PW], BF16, tag="ew")
                    e1w = ew[:, 0:PW]
                    e2w = ew[:, PW:2 * PW]
                    aw = a1_pool.tile([D, 2 * PW], BF16, tag="aw")
                    a1w = aw[:, 0:PW]
                    own = pools[c % 2]
                    for h in range(H):
                        stage1_half(rt, e1w, h, own)
                        if pending is None:
                            # group-granular Ln1 on the very first chunk:
                            # a1w of group h is ready right after its Exp,
                            # so mm2 + Exp2 of chunk 0 start 3 groups early.
                            nc.scalar.activation(
                                aw[:, h * G2:(h + 1) * G2],
                                ew[:, h * G2:(h + 1) * G2], f_ln, bias=1.0)
                        else:
                            stage2_half(pending[0], e2w, h, pending[2])
                    if pending is not None:
                        # Ln1(c) first, Ln2(prev) second: mm2 of chunk c
                        # (which needs a1(c)) starts under Ln2(prev), so the
                        # next chunk's Exp2 stream has no mm2 bubble.
                        nc.scalar.activation(
                            aw[:, 0:PW], ew[:, 0:PW], f_ln, bias=1.0)
                        nc.scalar.activation(
                            aw[:, PW:2 * PW], ew[:, PW:2 * PW],
                            f_ln, bias=1.0)
                        contract(aw[:, PW:2 * PW], pending[1], 0, H)
                    pending = (a1w, c * I_PAIR, own)
                return pending

            def flush(pending):
                """Drain the final chunk's layer 2 at group granularity
                (Ln2 FD=2048 + per-group contraction) so the tail after the
                final ACT op is one group's DVE work, not a whole chunk's."""
                ew = e1_pool.tile([D, 2 * PW], BF16, tag="ew")
                e2w = ew[:, PW:2 * PW]
                aw = a1_pool.tile([D, 2 * PW], BF16, tag="aw")
                i0 = pending[1]
                for h in range(H):
                    # both PSUM pools are free here: ping-pong so mm2(h+1)
                    # overlaps Exp2(h) instead of waiting for its read.
                    stage2_half(pending[0], e2w, h, pools[h % 2])
                    nc.scalar.activation(
                        aw[:, PW + h * G2:PW + (h + 1) * G2],
                        ew[:, PW + h * G2:PW + (h + 1) * G2], f_ln, bias=1.0)
                    contract(aw[:, PW:2 * PW], i0, h, 1)

            if reps == 1:
                flush(body(None))
            else:
                unroll = 1
                for u in (8, 4, 2):
                    if reps % u == 0:
                        unroll = u
                        break
                with tc.For_i(0, reps // unroll, 1):
                    p = None
                    for _ in range(unroll):
                        p = body(p)
                    flush(p)

            nc.sync.dma_start(outT[:], out_sb[:])

    nc.compile()
    return nc


def _prepare_inputs(x, r, W1, b1, W2, b2):
    bf16 = ml_dtypes.bfloat16
    W1 = np.asarray(W1, np.float32)
    W2 = np.asarray(W2, np.float32)
    w1s = np.concatenate([W1, W1], axis=0).astype(bf16)          # [128, 128]
    w2b = W2.astype(bf16)                                        # [128, 128]
    b1c = np.asarray(b1, np.float32).reshape(D, 1)
    b1sc = (A16 * np.asarray(b1, np.float64) + B16).astype(np.float32).reshape(D, 1)
    b2p = (np.asarray(b2, np.float32)
           - LOG2 * W2.sum(axis=0)).reshape(D, 1)

    in_maps = []
    for b in range(B):
        xbT = np.asarray(x[b], np.float32).T                     # [128 d, 256 j]
        in_maps.append({
            "rpt": np.ascontiguousarray(
                np.asarray(r[b], np.float32).reshape(PAIRS, 2 * RBF).T
            ).astype(bf16),
            "xte": np.ascontiguousarray(xbT[:, 0::2]).astype(bf16),
            "xto": np.ascontiguousarray(xbT[:, 1::2]).astype(bf16),
            "corr": (-LOG2 * xbT.sum(axis=1, dtype=np.float64)
                     ).astype(np.float32).reshape(D, 1),
            "w1s": w1s,
            "w2": w2b,
            "b1c": b1c,
            "b1s": b1sc,
            "b2p": b2p,
        })
    return in_maps


_NC_CACHE = None


def _get_nc():
    global _NC_CACHE
    if _NC_CACHE is None:
        _NC_CACHE = _build_program()
    return _NC_CACHE


def _make_runner(nc, in_maps):
    """Jitted PJRT runner with device-resident inputs (only the small
    donated output buffers are re-supplied per call), so repeated calls
    measure device time without the ~0.9 s host-transfer noise."""
    import jax
    from jax.sharding import Mesh, PartitionSpec
    from jax.experimental.shard_map import shard_map
    from concourse import bass2jax

    bass2jax.install_neuronx_cc_hook()
    partition_name = (
        nc.partition_id_tensor.name if nc.partition_id_tensor else None
    )
    in_names, out_names, out_avals, zero_outs = [], [], [], []
    for alloc in nc.m.functions[0].allocations:
        if not isinstance(alloc, mybir.MemoryLocationSet):
            continue
        name = alloc.memorylocations[0].name
        if alloc.kind == "ExternalInput":
            if name != partition_name:
                in_names.append(name)
        elif alloc.kind == "ExternalOutput":
            shape = tuple(alloc.tensor_shape)
            dtype = mybir.dt.np(alloc.dtype)
            out_names.append(name)
            import jax.core as _jc
            out_avals.append(_jc.ShapedArray(shape, dtype))
            zero_outs.append(np.zeros(shape, dtype))
    n_params = len(in_names)
    n_outs = len(out_avals)
    all_in_names = list(in_names) + list(out_names)
    if partition_name is not None:
        all_in_names.append(partition_name)
    donate = tuple(range(n_params, n_params + n_outs))

    def _body(*args):
        operands = list(args)
        if partition_name is not None:
            operands.append(bass2jax.partition_id_tensor())
        return tuple(bass2jax._bass_exec_p.bind(
            *operands,
            out_avals=tuple(out_avals),
            in_names=tuple(all_in_names),
            out_names=tuple(out_names),
            lowering_input_output_aliases=(),
            sim_require_finite=True,
            sim_require_nnan=True,
            nc=nc,
        ))

    devices = jax.devices()[:N_CORES]
    mesh = Mesh(np.asarray(devices), ("core",))
    sharded = jax.jit(
        shard_map(_body, mesh=mesh,
                  in_specs=(PartitionSpec("core"),) * (n_params + n_outs),
                  out_specs=(PartitionSpec("core"),) * len(out_names),
                  check_rep=False),
        donate_argnums=donate, keep_unused=True,
    )
    concat_in = [
        jax.device_put(np.concatenate(
            [np.asarray(in_maps[c][name]) for c in range(N_CORES)], axis=0))
        for name in in_names
    ]
    concat_zero = [
        np.zeros((N_CORES * z.shape[0], *z.shape[1:]), z.dtype)
        for z in zero_outs
    ]

    def call():
        import jax as _jax
        outs = sharded(*concat_in, *concat_zero)
        _jax.block_until_ready(outs)
        return outs

    return call


def hw_time_ns(inputs, reps=257, n_meas=7):
    """Measure on-device per-iteration time by comparing wall time of a
    reps-times device loop against a single-iteration run."""
    import time as _time
    in_maps = _prepare_inputs(**inputs)

    def min_wall(nc_prog):
        call = _make_runner(nc_prog, in_maps)
        call()  # warmup (compile/first exec)
        ts = []
        for _ in range(n_meas):
            t0 = _time.time()
            call()
            ts.append(_time.time() - t0)
        return min(ts)

    w1 = min_wall(_build_program(reps=1))
    wr = min_wall(_build_program(reps=reps))
    return (wr - w1) / (reps - 1) * 1e9


def kernel(x, r, W1, b1, W2, b2, _trace=False, _trace_kwargs=None):
    nc = _get_nc()
    in_maps = _prepare_inputs(x, r, W1, b1, W2, b2)
    res = run_bass_kernel_spmd(
        nc, in_maps, list(range(N_CORES)),
        trace=_trace, **(_trace_kwargs or {}),
    )
    out = np.stack([
        np.asarray(res.results[b]["outT"], np.float32).T for b in range(B)
    ])
    if _trace:
        return out, res
    return out


# revision 34
# speedup vs baseline: 1.0078x; 1.0078x over previous
"""CFConv (gnn message passing) Trainium2 kernel.

Math (per batch b):
    f1 = ssp(r @ W1 + b1)            ssp(x) = softplus(x) - log2
    f2 = ssp(f1 @ W2 + b2)
    out[i, d] = sum_j x[j, d] * f2[i, j, d]

Sharding: data-parallel over batch B=8 across the 8 cores (one batch each).

The kernel is ACT-bound: softplus = Ln(Exp(.)+1) costs two table passes
per element on the scalar engine (1 elem/lane/cycle at 1.2 GHz, no native
softplus table on this stack), twice per element = ~218 us/core of pure
ACT work.  Everything else (PE matmuls, DVE contraction, DMA) overlaps
under it.

Per-core device pipeline (features on partitions, (i,j) on the free dim):
  1. r is pre-transposed on the host to [128, PAIRS] bf16 (partitions
     0:64 hold the rbf features of even j, 64:128 of odd j), so the
     device does plain contiguous DMA loads (no xbar transpose).
  2. mm1: two K=64 row-tiled matmuls against a stacked [W1;W1] stationary
     (they run concurrently in different PE row groups).
  3. act1: Exp per 2048-col PSUM group then Ln per chunk; the "- log2"
     shift is folded into layer 2's bias (b2' below).  One of the four
     groups per chunk computes Exp on the otherwise-idle DVE instead,
     via the Schraudolph bits trick in bf16 space (one fused
     tensor_scalar: bits_i16 = round(A16*z1 + (A16*b1+B16)); the bit
     pattern IS exp(z1+b1) in bf16).  The exact ACT Ln that follows
     damps the sawtooth error by sigmoid(z1) and layer 2 stays exact:
     measured end-to-end rel err 0.0146 (budget 2e-2, fixed input seed).
     This takes a quarter of the Exp1 pass off the ACT bottleneck; the
     trick group runs last in the PSUM pool chain so its DVE convert
     hides under the Exp2 block and never stalls ACT.
  4. mm2: K=128 matmuls against W2.
  5. act2: exact two-pass softplus, bias b2' = b2 - log2 * sum_d W2[d,:].
  6. Contraction over j on DVE: prod = a2 * xT (xT broadcast over i),
     bf16 pair-halving add (TT at 2x) then 3D tensor_reduce over the
     inner j axis (1x), even+odd add, then a per-partition add of the
     "- log2 * sum_j x[j,d]" correction.  Output stays [d, i] on device;
     the host transposes back.

Scheduling: layer-2 work of chunk c-1 interleaves with layer-1 work of
chunk c at group granularity.  PSUM parity: chunk c uses pool c%2 for
both its layers, so neither matmul stream waits on the other chunk's Exp
reads.  Ln1(c) is emitted before Ln2(c-1) so mm2(c) starts under
Ln2(c-1).  For timing loops (reps > 1), eight full problem passes are
emitted per hardware-loop iteration and CHAINED (body k+1's chunk 0
interleaves with body k's last-chunk layer 2), so the per-iteration
all-engine barrier and the drain/ramp amortize over eight passes.
"""

import os
import tempfile

# The neuronxcc NEFF cache keys on the jit module interface, not the BIR
# content, so same-interface programs from other runs can serve stale
# NEFFs.  Give every process its own empty cache (compiles are ~2 s).
os.environ.setdefault(
    "NEURON_COMPILE_CACHE_URL", tempfile.mkdtemp(prefix="neff-cache-"))

import numpy as np
import ml_dtypes

import concourse.bass as bass
import concourse.tile as tile
from concourse import bacc, mybir
from concourse.bass_utils import run_bass_kernel_spmd

LOG2 = float(np.log(2.0))

B, N, D, RBF = 8, 256, 128, 64
PAIRS = N * N // 2            # 32768 row-pairs per batch
CHUNK_PAIRS = 4096            # pairs per DMA chunk (1 MiB)
GROUP_PAIRS = 1024            # pairs per PSUM group (8 query nodes i)
SUB = 512                     # pairs per matmul (one PSUM bank)
I_PER_GROUP = GROUP_PAIRS // (N // 2)   # 8
H = CHUNK_PAIRS // GROUP_PAIRS          # groups per chunk tile (4)
N_CORES = 8

BF16 = mybir.dt.bfloat16
F32 = mybir.dt.float32
I16 = mybir.dt.int16

# Schraudolph bf16-bits exp constants for the half-trick Exp1 path
A16 = 2.0 ** 7 / LOG2
C16 = 0.0579 * 2 ** 7
B16 = 127.0 * 2 ** 7 - C16


def _build_program(reps: int = 1):
    # Restrict the ACT-table chooser to the one set holding BOTH Exp and Ln;
    # otherwise it alternates between per-function sets and pays a ~2.7us
    # table load on every activation.
    import concourse.bacc as _bacc_mod
    from concourse.hw_specs import get_activation_tables as _gat
    _orig = _gat("gen3")
    _both = mybir.ActivationFunctionType.Exp, mybir.ActivationFunctionType.Ln
    _patched = {
        name: (funcs if name == "natural_log_exp_and_others"
               else type(funcs)(f for f in funcs if f not in _both))
        for name, funcs in _orig.items()
    }
    _bacc_mod.get_activation_tables = lambda arch: _patched

    nc = bacc.Bacc("TRN2", target_bir_lowering=False, debug=False,
                   num_devices=N_CORES)

    rpt = nc.dram_tensor("rpt", [2 * RBF, PAIRS], BF16, kind="ExternalInput").ap()
    xte = nc.dram_tensor("xte", [D, N // 2], BF16, kind="ExternalInput").ap()
    xto = nc.dram_tensor("xto", [D, N // 2], BF16, kind="ExternalInput").ap()
    corr = nc.dram_tensor("corr", [D, 1], F32, kind="ExternalInput").ap()
    w1s = nc.dram_tensor("w1s", [2 * RBF, D], BF16, kind="ExternalInput").ap()
    w2 = nc.dram_tensor("w2", [D, D], BF16, kind="ExternalInput").ap()
    b1 = nc.dram_tensor("b1c", [D, 1], F32, kind="ExternalInput").ap()
    b1s = nc.dram_tensor("b1s", [D, 1], F32, kind="ExternalInput").ap()
    b2p = nc.dram_tensor("b2p", [D, 1], F32, kind="ExternalInput").ap()
    outT = nc.dram_tensor("outT", [D, N], F32, kind="ExternalOutput").ap()

    f_exp = mybir.ActivationFunctionType.Exp
    f_ln = mybir.ActivationFunctionType.Ln
    mult = mybir.AluOpType.mult
    add = mybir.AluOpType.add

    with tile.TileContext(nc) as tc:
        with (
            tc.tile_pool(name="const", bufs=1) as const,
            tc.tile_pool(name="rt", bufs=6) as rt_pool,
            tc.tile_pool(name="e1", bufs=1) as e1_pool,
            tc.tile_pool(name="e2", bufs=1) as e2_pool,
            tc.tile_pool(name="a1", bufs=2) as a1_pool,
            tc.tile_pool(name="a2", bufs=2) as a2_pool,
            tc.tile_pool(name="prod", bufs=2) as prod_pool,
            tc.tile_pool(name="acc", bufs=2) as acc_pool,
            tc.tile_pool(name="osb", bufs=1) as out_pool,
            tc.tile_pool(name="f1", bufs=1, space="PSUM") as f1_pool,
            tc.tile_pool(name="f2", bufs=1, space="PSUM") as f2_pool,
        ):
            w1s_t = const.tile([2 * RBF, D], BF16, tag="w1s")
            w2_t = const.tile([D, D], BF16, tag="w2")
            xte_t = const.tile([D, N // 2], BF16, tag="xte")
            xto_t = const.tile([D, N // 2], BF16, tag="xto")
            b1_t = const.tile([D, 1], F32, tag="b1")
            b1s_t = const.tile([D, 1], F32, tag="b1s")
            b2p_t = const.tile([D, 1], F32, tag="b2p")
            corr_t = const.tile([D, 1], F32, tag="corr")
            # w1s/b1 first: they gate mm1/Exp1 of the first group.  The
            # first rt quarter-DMA is issued right after them (inside
            # body()) so it isn't queued behind the remaining consts.
            nc.sync.dma_start(w1s_t[:], w1s[:])
            nc.sync.dma_start(b1_t[:], b1[:])
            nc.sync.dma_start(b1s_t[:], b1s[:])

            out_sb = out_pool.tile([D, N], F32, tag="osb")

            def load_rest_of_consts():
                nc.sync.dma_start(w2_t[:], w2[:])
                nc.sync.dma_start(xte_t[:], xte[:])
                nc.sync.dma_start(xto_t[:], xto[:])
                nc.sync.dma_start(b2p_t[:], b2p[:])
                nc.sync.dma_start(corr_t[:], corr[:])

            # Tiny warmup activation right after the const loads: hoists the
            # ~2.7us ACT table load to t~0 where it overlaps the first DMA
            # instead of sitting in front of the first real Exp.
            warm = acc_pool.tile([D, 1], F32, tag="warm")
            nc.scalar.activation(warm[:], b1_t[:],
                                 mybir.ActivationFunctionType.Exp, bias=0.0)

            jw = N // 2
            G2 = 2 * GROUP_PAIRS          # cols per group (2048)
            PW = H * G2                   # cols per chunk tile (8192)
            I_PAIR = H * I_PER_GROUP      # 32 query nodes per chunk tile
            xe4 = xte_t[:, None, None, :].broadcast_to([D, H, I_PER_GROUP, jw])
            xo4 = xto_t[:, None, None, :].broadcast_to([D, H, I_PER_GROUP, jw])
            xe1 = xte_t[:, None, None, :].broadcast_to([D, 1, I_PER_GROUP, jw])
            xo1 = xto_t[:, None, None, :].broadcast_to([D, 1, I_PER_GROUP, jw])

            def stage1_half(rt, e1w, h, pool):
                """mm1 + Exp for one group (half pair)."""
                g0 = h * GROUP_PAIRS
                # f1 layout: [even 0:GROUP | odd GROUP:2*GROUP]
                f1 = pool.tile([D, G2], F32, tag="ps")
                for s in range(GROUP_PAIRS // SUB):
                    cs = g0 + s * SUB
                    nc.tensor.matmul(
                        f1[:, s * SUB:(s + 1) * SUB],
                        w1s_t[0:RBF, :],
                        rt[0:RBF, cs:cs + SUB],
                    )
                for s in range(GROUP_PAIRS // SUB):
                    cs = g0 + s * SUB
                    nc.tensor.matmul(
                        f1[:, GROUP_PAIRS + s * SUB:
                            GROUP_PAIRS + (s + 1) * SUB],
                        w1s_t[RBF:2 * RBF, :],
                        rt[RBF:2 * RBF, cs:cs + SUB],
                    )
                # softplus(z1 + b1) = Ln(Exp(z1 + b1) + 1).  Odd groups
                # compute Exp approximately on the (otherwise idle) DVE via
                # the Schraudolph bits trick in bf16 space -- one fused
                # tensor_scalar: bits_i16 = round(A16*z1 + (A16*b1 + B16)),
                # whose bit pattern IS exp(z1+b1) in bf16.  The exact ACT Ln
                # that follows damps the sawtooth error by sigmoid(z1), and
                # layer 2 stays exact; measured end-to-end rel err 0.0172
                # (budget 2e-2).  This moves half of Exp1 off the ACT
                # bottleneck (~15 us/body).
                if h == 3:
                    nc.vector.tensor_scalar(
                        out=e1w[:, h * G2:(h + 1) * G2].bitcast(I16),
                        in0=f1[:],
                        scalar1=float(A16),
                        scalar2=b1s_t[:],
                        op0=mult,
                        op1=add,
                    )
                else:
                    nc.scalar.activation(
                        e1w[:, h * G2:(h + 1) * G2], f1[:], f_exp,
                        bias=b1_t[:])

            def stage2_half(a1w, e2w, h, pool):
                """mm2 + Exp for one group of the previous chunk."""
                f2 = pool.tile([D, G2], F32, tag="ps")
                for s in range(G2 // SUB):
                    nc.tensor.matmul(
                        f2[:, s * SUB:(s + 1) * SUB],
                        w2_t[:],
                        a1w[:, h * G2 + s * SUB:h * G2 + (s + 1) * SUB],
                    )
                nc.scalar.activation(
                    e2w[:, h * G2:(h + 1) * G2], f2[:], f_exp, bias=b2p_t[:])

            def contract(a2w, i0, h0, nh):
                """weighted j-reduction of groups [h0, h0+nh) of a chunk.

                a2w cols = [h: [even 1024 | odd 1024]] * H; writes
                out_sb[:, i0 + h0*I_PER_GROUP : ... + nh*I_PER_GROUP].
                """
                cw = nh * G2
                c0 = h0 * G2
                iw = nh * I_PER_GROUP
                xe = xe4 if nh == H else xe1
                xo = xo4 if nh == H else xo1
                if nh not in (1, H):
                    xe = xte_t[:, None, None, :].broadcast_to(
                        [D, nh, I_PER_GROUP, jw])
                    xo = xto_t[:, None, None, :].broadcast_to(
                        [D, nh, I_PER_GROUP, jw])
                prod = prod_pool.tile([D, PW], BF16, tag="prod")
                p4 = prod[:, c0:c0 + cw].rearrange(
                    "p (h par k j) -> p h par k j", h=nh, par=2, j=jw)
                a4 = a2w[:, c0:c0 + cw].rearrange(
                    "p (h par k j) -> p h par k j", h=nh, par=2, j=jw)
                nc.vector.tensor_tensor(
                    p4[:, :, 0, :, :], a4[:, :, 0, :, :], xe, mult)
                nc.vector.tensor_tensor(
                    p4[:, :, 1, :, :], a4[:, :, 1, :, :], xo, mult)
                # Pair-halve in bf16 (TT runs 2x) before the 1x TensorReduce:
                # halves the reduce input, ~2.3us/chunk less DVE time.
                m = 2 * iw      # 16 per group: (par, k)
                ph = prod_pool.tile([D, PW // 2], BF16, tag="ph")
                p3 = prod[:, c0:c0 + cw].rearrange(
                    "p (m half j) -> p m half j", m=m, half=2)
                nc.vector.tensor_tensor(
                    ph[:, 0:cw // 2].rearrange("p (m j) -> p m j", m=m),
                    p3[:, :, 0, :], p3[:, :, 1, :], add)
                # sums index m = h*16 + par*8 + k
                sums = acc_pool.tile([D, 2 * I_PAIR], F32, tag="sums")
                nc.vector.tensor_reduce(
                    sums[:, 0:2 * iw],
                    ph[:, 0:cw // 2].rearrange("p (m j) -> p m j", m=m),
                    axis=mybir.AxisListType.X,
                    op=add,
                )
                tmp = acc_pool.tile([D, I_PAIR], F32, tag="tmp")
                s4 = sums[:, 0:2 * iw].rearrange(
                    "p (h par k) -> p h par k", h=nh, par=2)
                nc.vector.tensor_add(
                    tmp[:, 0:iw].rearrange("p (h k) -> p h k", h=nh),
                    s4[:, :, 0, :], s4[:, :, 1, :])
                nc.vector.tensor_scalar_add(
                    out_sb[:, i0 + h0 * I_PER_GROUP:
                           i0 + h0 * I_PER_GROUP + iw],
                    tmp[:, 0:iw], corr_t[:])

            # Software-pipelined emission interleaving halves of chunk c's
            # layer 1 with halves of chunk c-1's layer 2, so every ACT op has
            # a PE window in front of it and ACT never head-of-line blocks.
            # PSUM parity: chunk c uses pool c%2 for BOTH its layers (its
            # layer 2 runs interleaved with chunk c+1's layer 1, which owns
            # the other pool), so neither matmul stream ever waits on the
            # other chunk's Exp reads.
            pools = (f1_pool, f2_pool)

            n_chunks = PAIRS // CHUNK_PAIRS

            def body(pending):
                """One full problem pass.  `pending` is the (a1w, i0, pool)
                of the previous body's last chunk: its layer 2 interleaves
                with this body's chunk 0 exactly like an ordinary chunk
                boundary, so chained bodies pipeline seamlessly."""
                first = pending is None
                for c in range(n_chunks):
                    rt = rt_pool.tile([2 * RBF, CHUNK_PAIRS], BF16, tag="rt")
                    if first and c == 0:
                        # Quarter the first load so mm1 of group 0 can
                        # start as soon as the first 256 KiB lands.  The
                        # remaining const DMAs queue behind quarter 0.
                        q = CHUNK_PAIRS // 4
                        nc.sync.dma_start(rt[:, 0:q], rpt[:, 0:q])
                        load_rest_of_consts()
                        for k in range(1, 4):
                            nc.sync.dma_start(
                                rt[:, k * q:(k + 1) * q],
                                rpt[:, k * q:(k + 1) * q],
                            )
                    elif first and c == 1:
                        hq = CHUNK_PAIRS // 2
                        for k in range(2):
                            nc.sync.dma_start(
                                rt[:, k * hq:(k + 1) * hq],
                                rpt[:, c * CHUNK_PAIRS + k * hq:
                                    c * CHUNK_PAIRS + (k + 1) * hq],
                            )
                    else:
                        nc.sync.dma_start(
                            rt[:],
                            rpt[:, c * CHUNK_PAIRS:(c + 1) * CHUNK_PAIRS],
                        )
                    # e1 (chunk c) and e2 (previous chunk) share ONE tile;
                    # their Ln passes run back to back from it.
                    ew = e1_pool.tile([D, 2 * PW], BF16, tag="ew")
                    e1w = ew[:, 0:PW]
                    e2w = ew[:, PW:2 * PW]
                    aw = a1_pool.tile([D, 2 * PW], BF16, tag="aw")
                    a1w = aw[:, 0:PW]
                    own = pools[c % 2]
                    for h in range(H):
                        stage1_half(rt, e1w, h, own)
                        if pending is None:
                            # group-granular Ln1 on the very first chunk:
                            # a1w of group h is ready right after its Exp,
                            # so mm2 + Exp2 of chunk 0 start 3 groups early.
                            nc.scalar.activation(
                                aw[:, h * G2:(h + 1) * G2],
                                ew[:, h * G2:(h + 1) * G2], f_ln, bias=1.0)
                        else:
                            stage2_half(pending[0], e2w, h, pending[2])
                    if pending is not None:
                        # Ln1(c) first, Ln2(prev) second: mm2 of chunk c
                        # (which needs a1(c)) starts under Ln2(prev), so the
                        # next chunk's Exp2 stream has no mm2 bubble.
                        nc.scalar.activation(
                            aw[:, 0:PW], ew[:, 0:PW], f_ln, bias=1.0)
                        nc.scalar.activation(
                            aw[:, PW:2 * PW], ew[:, PW:2 * PW],
                            f_ln, bias=1.0)
                        contract(aw[:, PW:2 * PW], pending[1], 0, H)
                    pending = (a1w, c * I_PAIR, own)
                return pending

            def flush(pending):
                """Drain the final chunk's layer 2 at group granularity
                (Ln2 FD=2048 + per-group contraction) so the tail after the
                final ACT op is one group's DVE work, not a whole chunk's."""
                ew = e1_pool.tile([D, 2 * PW], BF16, tag="ew")
                e2w = ew[:, PW:2 * PW]
                aw = a1_pool.tile([D, 2 * PW], BF16, tag="aw")
                i0 = pending[1]
                for h in range(H):
                    # both PSUM pools are free here: ping-pong so mm2(h+1)
                    # overlaps Exp2(h) instead of waiting for its read.
                    stage2_half(pending[0], e2w, h, pools[h % 2])
                    nc.scalar.activation(
                        aw[:, PW + h * G2:PW + (h + 1) * G2],
                        ew[:, PW + h * G2:PW + (h + 1) * G2], f_ln, bias=1.0)
                    contract(aw[:, PW:2 * PW], i0, h, 1)

            if reps == 1:
                flush(body(None))
            else:
                unroll = 1
                for u in (8, 4, 2):
                    if reps % u == 0:
                        unroll = u
                        break
                with tc.For_i(0, reps // unroll, 1):
                    p = None
                    for _ in range(unroll):
                        p = body(p)
                    flush(p)

            nc.sync.dma_start(outT[:], out_sb[:])

    nc.compile()
    return nc


def _prepare_inputs(x, r, W1, b1, W2, b2):
    bf16 = ml_dtypes.bfloat16
    W1 = np.asarray(W1, np.float32)
    W2 = np.asarray(W2, np.float32)
    w1s = np.concatenate([W1, W1], axis=0).astype(bf16)          # [128, 128]
    w2b = W2.astype(bf16)                                        # [128, 128]
    b1c = np.asarray(b1, np.float32).reshape(D, 1)
    b1sc = (A16 * np.asarray(b1, np.float64) + B16).astype(np.float32).reshape(D, 1)
    b2p = (np.asarray(b2, np.float32)
           - LOG2 * W2.sum(axis=0)).reshape(D, 1)

    in_maps = []
    for b in range(B):
        xbT = np.asarray(x[b], np.float32).T                     # [128 d, 256 j]
        in_maps.append({
            "rpt": np.ascontiguousarray(
                np.asarray(r[b], np.float32).reshape(PAIRS, 2 * RBF).T
            ).astype(bf16),
            "xte": np.ascontiguousarray(xbT[:, 0::2]).astype(bf16),
            "xto": np.ascontiguousarray(xbT[:, 1::2]).astype(bf16),
            "corr": (-LOG2 * xbT.sum(axis=1, dtype=np.float64)
                     ).astype(np.float32).reshape(D, 1),
            "w1s": w1s,
            "w2": w2b,
            "b1c": b1c,
            "b1s": b1sc,
            "b2p": b2p,
        })
    return in_maps


_NC_CACHE = None


def _get_nc():
    global _NC_CACHE
    if _NC_CACHE is None:
        _NC_CACHE = _build_program()
    return _NC_CACHE


def _make_runner(nc, in_maps):
    """Jitted PJRT runner with device-resident inputs (only the small
    donated output buffers are re-supplied per call), so repeated calls
    measure device time without the ~0.9 s host-transfer noise."""
    import jax
    from jax.sharding import Mesh, PartitionSpec
    from jax.experimental.shard_map import shard_map
    from concourse import bass2jax

    bass2jax.install_neuronx_cc_hook()
    partition_name = (
        nc.partition_id_tensor.name if nc.partition_id_tensor else None
    )
    in_names, out_names, out_avals, zero_outs = [], [], [], []
    for alloc in nc.m.functions[0].allocations:
        if not isinstance(alloc, mybir.MemoryLocationSet):
            continue
        name = alloc.memorylocations[0].name
        if alloc.kind == "ExternalInput":
            if name != partition_name:
                in_names.append(name)
        elif alloc.kind == "ExternalOutput":
            shape = tuple(alloc.tensor_shape)
            dtype = mybir.dt.np(alloc.dtype)
            out_names.append(name)
            import jax.core as _jc
            out_avals.append(_jc.ShapedArray(shape, dtype))
            zero_outs.append(np.zeros(shape, dtype))
    n_params = len(in_names)
    n_outs = len(out_avals)
    all_in_names = list(in_names) + list(out_names)
    if partition_name is not None:
        all_in_names.append(partition_name)
    donate = tuple(range(n_params, n_params + n_outs))

    def _body(*args):
        operands = list(args)
        if partition_name is not None:
            operands.append(bass2jax.partition_id_tensor())
        return tuple(bass2jax._bass_exec_p.bind(
            *operands,
            out_avals=tuple(out_avals),
            in_names=tuple(all_in_names),
            out_names=tuple(out_names),
            lowering_input_output_aliases=(),
            sim_require_finite=True,
            sim_require_nnan=True,
            nc=nc,
        ))

    devices = jax.devices()[:N_CORES]
    mesh = Mesh(np.asarray(devices), ("core",))
    sharded = jax.jit(
        shard_map(_body, mesh=mesh,
                  in_specs=(PartitionSpec("core"),) * (n_params + n_outs),
                  out_specs=(PartitionSpec("core"),) * len(out_names),
                  check_rep=False),
        donate_argnums=donate, keep_unused=True,
    )
    concat_in = [
        jax.device_put(np.concatenate(
            [np.asarray(in_maps[c][name]) for c in range(N_CORES)], axis=0))
        for name in in_names
    ]
    concat_zero = [
        np.zeros((N_CORES * z.shape[0], *z.shape[1:]), z.dtype)
        for z in zero_outs
    ]

    def call():
        import jax as _jax
        outs = sharded(*concat_in, *concat_zero)
        _jax.block_until_ready(outs)
        return outs

    return call


def hw_time_ns(inputs, reps=257, n_meas=7):
    """Measure on-device per-iteration time by comparing wall time of a
    reps-times device loop against a single-iteration run."""
    import time as _time
    in_maps = _prepare_inputs(**inputs)

    def min_wall(nc_prog):
        call = _make_runner(nc_prog, in_maps)
        call()  # warmup (compile/first exec)
        ts = []
        for _ in range(n_meas):
            t0 = _time.time()
            call()
            ts.append(_time.time() - t0)
        return min(ts)

    w1 = min_wall(_build_program(reps=1))
    wr = min_wall(_build_program(reps=reps))
    return (wr - w1) / (reps - 1) * 1e9


def kernel(x, r, W1, b1, W2, b2, _trace=False, _trace_kwargs=None):
    nc = _get_nc()
    in_maps = _prepare_inputs(x, r, W1, b1, W2, b2)
    res = run_bass_kernel_spmd(
        nc, in_maps, list(range(N_CORES)),
        trace=_trace, **(_trace_kwargs or {}),
    )
    out = np.stack([
        np.asarray(res.results[b]["outT"], np.float32).T for b in range(B)
    ])
    if _trace:
        return out, res
    return out
